# revision 2
# baseline (speedup 1.0000x reference)
"""Trainium2 Bass kernel for nn_ALNet (adaptive linear network forward).

Math: vals = x @ W + b  ([B,256] @ [256,128] + [128]), then a 7-level
alternating min/max pairwise tree over the 128 leaf columns -> [B, 1].

Strategy (8 NeuronCores, data-parallel over batch):
  - Host: transpose each core's batch shard to xT [256, 8192] so the
    contraction dim (features) lands on SBUF partitions with zero on-device
    transposes; bit-reverse-permute W's columns (and b) so the alternating
    min/max tree becomes 7 contiguous half-vs-half tensor_tensor ops.
  - Device per core: PE computes psum[batch_tile, leaves] via
    LDW(xT chunk) + MM(W k-tile) pairs (K=256 -> 2 accumulating matmuls);
    DVE evicts PSUM with a fused bias-add (tensor_tensor add vs a
    broadcast-bias tile) casting to bf16, then runs the 7 halving
    min/max levels batched across 16 batch-tiles per op.
  - Output staged as [128, 64] (out[p, c] = result for batch row 128*c+p),
    un-permuted on host.
"""

import numpy as np

try:
    import concourse.bass as bass
except ImportError:  # pragma: no cover
    import sys

    sys.path.insert(0, "/opt/trn_rl_repo")
    import concourse.bass as bass

import concourse.mybir as mybir
import concourse.tile as tile
from concourse import bacc
from concourse.bass_utils import run_bass_kernel_spmd

F32 = mybir.dt.float32
BF16 = mybir.dt.bfloat16

B, F, NL = 65536, 256, 128
NCORES = 8
BS = B // NCORES  # 8192 batch rows per core

# Tree ops, deepest level first (reference folds reversed root->leaf list;
# the list [min,max,min,...] of length 7 is a palindrome).
_TREE_OPS = [
    mybir.AluOpType.min if i % 2 == 0 else mybir.AluOpType.max for i in range(7)
]


def _bitrev7_perm() -> np.ndarray:
    perm = np.zeros(NL, dtype=np.int64)
    for p in range(NL):
        r = 0
        for k in range(7):
            r |= ((p >> k) & 1) << (6 - k)
        perm[p] = r
    return perm


def build_nc(bs: int = BS, chunk: int = 2048, val_dt=BF16):
    """Build the per-core Bass graph. bs = batch rows per core; chunk =
    batch columns processed per group (multiple of 128, <= 2048)."""
    assert bs % chunk == 0 and chunk % 128 == 0
    ng = bs // chunk  # groups
    tpb = chunk // 128  # 128-row batch tiles per group
    ncols = bs // 128  # out staging columns

    nc = bacc.Bacc(None)
    xT = nc.declare_dram_parameter("xT", [F, bs], F32, isOutput=False)
    Wp = nc.declare_dram_parameter("Wp", [F, NL], F32, isOutput=False)
    bb = nc.declare_dram_parameter("bb", [128, 512], F32, isOutput=False)
    out = nc.declare_dram_parameter("out", [128, ncols], F32, isOutput=True)

    with tile.TileContext(nc) as tc:
        with (
            tc.tile_pool(name="const", bufs=1) as cpool,
            tc.tile_pool(name="xin", bufs=2) as xpool,
            tc.tile_pool(name="psum", bufs=2, space=bass.MemorySpace.PSUM) as ppool,
            tc.tile_pool(name="vals", bufs=2) as vpool,
            tc.tile_pool(name="lvl", bufs=2) as lpool,
            tc.tile_pool(name="ostage", bufs=1) as opool,
        ):
            w0 = cpool.tile([128, NL], F32, tag="w0")
            w1 = cpool.tile([128, NL], F32, tag="w1")
            nc.sync.dma_start(out=w0[:], in_=Wp[0:128, :])
            nc.sync.dma_start(out=w1[:], in_=Wp[128:256, :])

            bias = cpool.tile([128, chunk], F32, tag="bias")
            nc.sync.dma_start(out=bias[:, 0 : min(512, chunk)], in_=bb[:, 0:chunk] if chunk < 512 else bb[:])
            for q in range(1, chunk // 512):
                nc.vector.tensor_copy(
                    out=bias[:, q * 512 : (q + 1) * 512], in_=bias[:, 0:512]
                )

            ost = opool.tile([128, ncols], F32, tag="ost")

            for g in range(ng):
                x0 = xpool.tile([128, chunk], F32, tag="x0")
                x1 = xpool.tile([128, chunk], F32, tag="x1")
                c0 = g * chunk
                nc.sync.dma_start(out=x0[:], in_=xT[0:128, c0 : c0 + chunk])
                nc.sync.dma_start(out=x1[:], in_=xT[128:256, c0 : c0 + chunk])

                ps = ppool.tile([128, chunk], F32, tag="ps")
                for t in range(tpb):
                    sl = bass.ts(t, 128)
                    nc.tensor.matmul(
                        ps[:, sl], x0[:, sl], w0[:], start=True, stop=False
                    )
                    nc.tensor.matmul(
                        ps[:, sl], x1[:, sl], w1[:], start=False, stop=True
                    )

                # PSUM eviction fused with bias add (and cast to val_dt)
                v = vpool.tile([128, chunk], val_dt, tag="v")
                nc.vector.tensor_tensor(
                    out=v[:], in0=ps[:], in1=bias[:], op=mybir.AluOpType.add
                )

                # 7 halving tree levels, batched across the tpb blocks
                cur = v
                w = NL // 2  # half-width within each block
                for lvl, op in enumerate(_TREE_OPS):
                    r = cur[:].rearrange("p (blk two h) -> p blk two h", two=2, h=w)
                    in0 = r[:, :, 0, :]
                    in1 = r[:, :, 1, :]
                    if lvl < 6:
                        nxt = lpool.tile([128, tpb * w], val_dt, tag=f"lvl{lvl}")
                        outap = nxt[:].rearrange("p (blk h) -> p blk h", h=w)
                    else:
                        nxt = None
                        outap = ost[:, g * tpb : (g + 1) * tpb].rearrange(
                            "p (blk h) -> p blk h", h=1
                        )
                    nc.vector.tensor_tensor(out=outap, in0=in0, in1=in1, op=op)
                    cur = nxt
                    w //= 2

            nc.sync.dma_start(out=out[:], in_=ost[:])

    nc.compile()
    return nc


_NC_CACHE: dict = {}


def _get_nc(bs=BS, chunk=2048):
    key = (bs, chunk)
    if key not in _NC_CACHE:
        _NC_CACHE[key] = build_nc(bs, chunk)
    return _NC_CACHE[key]


def prep_inputs(x: np.ndarray, W: np.ndarray, b: np.ndarray) -> list[dict]:
    perm = _bitrev7_perm()
    Wp = np.ascontiguousarray(W[:, perm], dtype=np.float32)
    bp = np.ascontiguousarray(b[perm], dtype=np.float32)
    bb = np.ascontiguousarray(np.tile(bp[None, :], (128, 4)))  # [128, 512]
    in_maps = []
    for i in range(NCORES):
        xTi = np.ascontiguousarray(x[i * BS : (i + 1) * BS, :].T, dtype=np.float32)
        in_maps.append({"xT": xTi, "Wp": Wp, "bb": bb})
    return in_maps


def gather_outputs(results: list[dict]) -> np.ndarray:
    shards = []
    for i in range(NCORES):
        o = np.asarray(results[i]["out"])  # [128, BS//128]; o[p, c] = row 128c+p
        shards.append(o.T.reshape(BS))
    return np.concatenate(shards).reshape(B, 1).astype(np.float32)


def _setup_tracing():
    """Install the antenv.axon_hooks NTFF-profile shim (missing from this
    image) and neuter the artifact upload so traced runs stay local."""
    import sys as _sys
    import types

    import concourse.bass_utils as bu

    bu.upload_artifacts = lambda tmpdir: tmpdir
    try:
        from antenv.axon_hooks import get_axon_ntff_profile_hook  # noqa: F401

        return
    except ImportError:
        pass
    import antenv

    m = types.ModuleType("antenv.axon_hooks")
    _state = {"hook": None}
    m.set_axon_ntff_profile_hook = lambda h: _state.__setitem__("hook", h)
    m.get_axon_ntff_profile_hook = lambda: _state["hook"]
    _sys.modules["antenv.axon_hooks"] = m
    antenv.axon_hooks = m
    try:
        from trn_agent_boot.trn_boot import _ntff_profile_via_ctypes

        hook = _ntff_profile_via_ctypes("/opt/axon/libaxon_pjrt.so")
        if hook is not None:
            m.set_axon_ntff_profile_hook(hook)
    except Exception as e:  # pragma: no cover
        print("ntff hook install failed:", e)


def run_on_hw(x, W, b, trace: bool = False, **kwargs):
    if trace:
        _setup_tracing()
    nc = _get_nc()
    in_maps = prep_inputs(np.asarray(x), np.asarray(W), np.asarray(b))
    return run_bass_kernel_spmd(
        nc, in_maps, core_ids=list(range(NCORES)), trace=trace, **kwargs
    )


def kernel(x: np.ndarray, W: np.ndarray, b: np.ndarray) -> np.ndarray:
    res = run_on_hw(x, W, b, trace=False)
    return gather_outputs(res.results)


# revision 5
# speedup vs baseline: 1.6030x; 1.6030x over previous
"""Trainium2 Bass kernel for nn_ALNet (adaptive linear network forward).

Math: vals = x @ W + b  ([B,256] @ [256,128] + [128]), then a 7-level
alternating min/max pairwise tree over the 128 leaf columns -> [B, 1].

Strategy (8 NeuronCores, data-parallel over batch):
  - Host: transpose each core's batch shard to xT [256, 8192] (bf16) so the
    contraction dim lands on SBUF partitions with zero on-device transposes;
    bit-reverse-permute W's columns (and b) so the alternating min/max tree
    becomes 7 contiguous half-vs-half tensor_tensor ops.
  - Device per core, per group of 2048 batch rows:
      PE:  bias seeded via rank-1 ones x bias_row matmuls (start=True),
           then x @ W accumulated as LDW(x tile)+MM(W k-tile) pairs in bf16
           (all K-half-0 MMs emitted before K-half-1 so PE starts as soon
           as the first half-DMA lands).
      ACT: evicts PSUM f32 -> SBUF bf16 (copy).
      DVE: 7 halving min/max levels batched across 16 batch-tiles.
  - Output staged as [128, 64] f32 (out[p, c] = batch row 128*c+p),
    de-interleaved on host.
"""

import numpy as np

try:
    import concourse.bass as bass
except ImportError:  # pragma: no cover
    import sys

    sys.path.insert(0, "/opt/trn_rl_repo")
    import concourse.bass as bass

import ml_dtypes
import concourse.mybir as mybir
import concourse.tile as tile
from concourse import bacc
from concourse.bass_utils import run_bass_kernel_spmd

F32 = mybir.dt.float32
BF16 = mybir.dt.bfloat16

B, F, NL = 65536, 256, 128
NCORES = 8
BS = B // NCORES  # 8192 batch rows per core

# Tree ops, deepest level first (reference folds reversed root->leaf list;
# the list [min,max,min,...] of length 7 is a palindrome).
_TREE_OPS = [
    mybir.AluOpType.min if i % 2 == 0 else mybir.AluOpType.max for i in range(7)
]


def _bitrev7_perm() -> np.ndarray:
    perm = np.zeros(NL, dtype=np.int64)
    for p in range(NL):
        r = 0
        for k in range(7):
            r |= ((p >> k) & 1) << (6 - k)
        perm[p] = r
    return perm


def build_nc(bs: int = BS, chunk: int = 2048, sup: int = 4096):
    """bs = batch rows per core; chunk = rows per PSUM group (<=2048,
    mult of 128); sup = rows per x super-load (mult of chunk)."""
    assert bs % sup == 0 and sup % chunk == 0 and chunk % 128 == 0
    nsup = bs // sup
    gps = sup // chunk  # groups per super-load
    tpb = chunk // 128  # 128-row batch tiles per group
    ncols = bs // 128

    nc = bacc.Bacc(None)
    xT = nc.declare_dram_parameter("xT", [F, bs], BF16, isOutput=False)
    Wp = nc.declare_dram_parameter("Wp", [F, NL], BF16, isOutput=False)
    brow = nc.declare_dram_parameter("brow", [1, 512], BF16, isOutput=False)
    ones = nc.declare_dram_parameter("ones", [1, 128], BF16, isOutput=False)
    out = nc.declare_dram_parameter("out", [128, ncols], F32, isOutput=True)

    with tile.TileContext(nc) as tc:
        with (
            tc.tile_pool(name="const", bufs=1) as cpool,
            tc.tile_pool(name="xin", bufs=2) as xpool,
            tc.tile_pool(name="psum", bufs=2, space=bass.MemorySpace.PSUM) as ppool,
            tc.tile_pool(name="vals", bufs=2) as vpool,
            tc.tile_pool(name="lvl", bufs=2) as lpool,
            tc.tile_pool(name="ostage", bufs=1) as opool,
        ):
            w0 = cpool.tile([128, NL], BF16, tag="w0")
            w1 = cpool.tile([128, NL], BF16, tag="w1")
            nc.sync.dma_start(out=w0[:], in_=Wp[0:128, :])
            nc.sync.dma_start(out=w1[:], in_=Wp[128:256, :])
            br = cpool.tile([1, 512], BF16, tag="brow")
            on = cpool.tile([1, 128], BF16, tag="ones")
            nc.sync.dma_start(out=br[:], in_=brow[:])
            nc.sync.dma_start(out=on[:], in_=ones[:])

            ost = opool.tile([128, ncols], F32, tag="ost")

            for s in range(nsup):
                x0 = xpool.tile([128, sup], BF16, tag="x0")
                x1 = xpool.tile([128, sup], BF16, tag="x1")
                s0 = s * sup
                nc.sync.dma_start(out=x0[:], in_=xT[0:128, s0 : s0 + sup])
                nc.sync.dma_start(out=x1[:], in_=xT[128:256, s0 : s0 + sup])

                for g in range(gps):
                    c0 = g * chunk  # offset within the super-load
                    ps = ppool.tile([128, chunk], F32, tag="ps")
                    # seed bias: ones^T @ brow broadcasts brow over batch rows.
                    # One start=True MM per PSUM bank (zero-region); the last
                    # accumulating MM in each bank carries stop=True.
                    for bank in range(chunk // 512):
                        nc.tensor.matmul(
                            ps[:, bass.ts(bank, 512)],
                            on[:],
                            br[:],
                            start=True,
                            stop=False,
                        )
                    # K half 0 for all tiles, then K half 1
                    for t in range(tpb):
                        sl = bass.ts(t, 128)
                        xsl = bass.ds(c0 + t * 128, 128)
                        nc.tensor.matmul(
                            ps[:, sl], x0[:, xsl], w0[:], start=False, stop=False
                        )
                    for t in range(tpb):
                        sl = bass.ts(t, 128)
                        xsl = bass.ds(c0 + t * 128, 128)
                        nc.tensor.matmul(
                            ps[:, sl],
                            x1[:, xsl],
                            w1[:],
                            start=False,
                            stop=(t % 4 == 3),
                        )

                    # ACT evicts PSUM f32 -> SBUF bf16
                    v = vpool.tile([128, chunk], BF16, tag="v")
                    nc.scalar.copy(out=v[:], in_=ps[:])

                    # 7 halving tree levels on DVE, batched across tpb blocks
                    cur = v
                    w = NL // 2
                    gi = s * gps + g
                    for lvl, op in enumerate(_TREE_OPS):
                        r = cur[:].rearrange(
                            "p (blk two h) -> p blk two h", two=2, h=w
                        )
                        in0 = r[:, :, 0, :]
                        in1 = r[:, :, 1, :]
                        if lvl < 6:
                            nxt = lpool.tile([128, tpb * w], BF16, tag=f"lvl{lvl}")
                            outap = nxt[:].rearrange("p (blk h) -> p blk h", h=w)
                        else:
                            nxt = None
                            outap = ost[:, gi * tpb : (gi + 1) * tpb].rearrange(
                                "p (blk h) -> p blk h", h=1
                            )
                        nc.vector.tensor_tensor(out=outap, in0=in0, in1=in1, op=op)
                        cur = nxt
                        w //= 2

            nc.sync.dma_start(out=out[:], in_=ost[:])

    nc.compile()
    return nc


_NC_CACHE: dict = {}


def _get_nc(bs=BS, chunk=2048, sup=4096):
    key = (bs, chunk, sup)
    if key not in _NC_CACHE:
        _NC_CACHE[key] = build_nc(bs, chunk, sup)
    return _NC_CACHE[key]


def prep_inputs(x: np.ndarray, W: np.ndarray, b: np.ndarray) -> list[dict]:
    perm = _bitrev7_perm()
    bf = ml_dtypes.bfloat16
    Wp = np.ascontiguousarray(W[:, perm]).astype(bf)
    bp = np.ascontiguousarray(b[perm]).astype(bf)
    brow = np.ascontiguousarray(np.tile(bp[None, :], (1, 4)))  # [1, 512]
    ones = np.ones((1, 128), dtype=bf)
    x = np.asarray(x, dtype=np.float32)
    in_maps = []
    for i in range(NCORES):
        xTi = np.ascontiguousarray(x[i * BS : (i + 1) * BS, :].T).astype(bf)
        in_maps.append({"xT": xTi, "Wp": Wp, "brow": brow, "ones": ones})
    return in_maps


def gather_outputs(results: list[dict]) -> np.ndarray:
    shards = []
    for i in range(NCORES):
        o = np.asarray(results[i]["out"])  # [128, BS//128]; o[p, c] = row 128c+p
        shards.append(o.T.reshape(BS))
    return np.concatenate(shards).reshape(B, 1).astype(np.float32)


def _setup_tracing():
    """Install the antenv.axon_hooks NTFF-profile shim (missing from this
    image) and neuter the artifact upload so traced runs stay local."""
    import sys as _sys
    import types

    import concourse.bass_utils as bu

    bu.upload_artifacts = lambda tmpdir: tmpdir
    try:
        from antenv.axon_hooks import get_axon_ntff_profile_hook  # noqa: F401

        return
    except ImportError:
        pass
    import antenv

    m = types.ModuleType("antenv.axon_hooks")
    _state = {"hook": None}
    m.set_axon_ntff_profile_hook = lambda h: _state.__setitem__("hook", h)
    m.get_axon_ntff_profile_hook = lambda: _state["hook"]
    _sys.modules["antenv.axon_hooks"] = m
    antenv.axon_hooks = m
    try:
        from trn_agent_boot.trn_boot import _ntff_profile_via_ctypes

        hook = _ntff_profile_via_ctypes("/opt/axon/libaxon_pjrt.so")
        if hook is not None:
            m.set_axon_ntff_profile_hook(hook)
    except Exception as e:  # pragma: no cover
        print("ntff hook install failed:", e)


def run_on_hw(x, W, b, trace: bool = False, **kwargs):
    if trace:
        _setup_tracing()
    nc = _get_nc()
    in_maps = prep_inputs(np.asarray(x), np.asarray(W), np.asarray(b))
    return run_bass_kernel_spmd(
        nc, in_maps, core_ids=list(range(NCORES)), trace=trace, **kwargs
    )


def kernel(x: np.ndarray, W: np.ndarray, b: np.ndarray) -> np.ndarray:
    res = run_on_hw(x, W, b, trace=False)
    return gather_outputs(res.results)
